# revision 17
# baseline (speedup 1.0000x reference)
"""Trainium2 Bass kernel for masked GAT-style attention softmax.

reference: softmax(where(mask, -1e9, leakyrelu(s1[:,None]+s2[None,:])), -1)
with s1 = x@w1, s2 = x@w2.  B=8 batches -> data-parallel over 8 NeuronCores.

Per-core layout [i_part, j_free], fp16 compute / f32 accums.  The ACT
engine's exp lookup table is rebuilt at compile time so that for x < 0 it
evaluates exp(alpha*x) instead of exp(x) -- i.e. Exp becomes
H(x) = exp(leakyrelu(x)) in a single table pass (the PWP bucket tables
store per-segment Taylor coefficients [d0..d3, x0]; only the negative
segments' coefficients change, positives and saturation stay stock).

Per tile [128, 4096]:
  DVE : w = m16 + s2b            (tensor_tensor, 2x mode, fp16)
  ACT : p = H(w + s1[i]), rowsum (hijacked Exp, per-partition bias, accum)
  DVE : r -> 1/r (reciprocal), out = p * (1/r) (tensor_scalar, 4x mode)
ACT is the bottleneck at ~137us/core; DVE ~128us.  GpSimd only issues
output DMAs (Pool tensor ops measured 10-50x below the cost model on HW
and they stall DVE via the shared SBUF ports -- keep Pool off compute).
Masks + prologue loads issue on Sync; mask issues run LEAD tiles ahead.
s2b comes from a stride-0 broadcast DMA readback of the s12 projection
row.  Host pre-bakes the fp16 {-100, 0} mask fill."""

import numpy as np

B, N, F = 8, 4096, 256
P = 128
NT = N // P  # 32 row tiles per core
MASKC = -100.0
ALPHA = 0.2


def _make_hijacked_act_root():
    """Build a patched copy of the neuronxcc PWP activation tables where
    exp's negative-x bucket entries hold Taylor coefficients of
    exp(ALPHA*x), so ActivationFunctionType.Exp computes exp(leakyrelu(x)).
    Returns the path to the patched act_info.json (cached per-process)."""
    import hashlib
    import json
    import os
    import shutil
    from pathlib import Path

    if _CUSTOM.get("act_root"):
        return _CUSTOM["act_root"]

    from neuronxcc.driver.Job import Job

    pkg = Path(Job.getPackageDir())
    src_dir = None
    for cand in ("pwp_bin_trainium",):
        if (pkg / "pwp" / cand / "act_info.json").exists():
            src_dir = pkg / "pwp" / cand
    if src_dir is None:
        from neuronxcc.driver.jobs.support.FindActInfo import findActInfoFile

        src_dir = Path(findActInfoFile(str(pkg), "gen3")).parent

    tag = hashlib.md5(
        f"lrelu-exp-{ALPHA}-{src_dir}".encode()
    ).hexdigest()[:10]
    dst = Path(os.environ.get("TMPDIR", "/tmp")) / f"bass_act_lrelu_{tag}"
    info_path = dst / "act_info.json"
    if not info_path.exists():
        tmp = Path(str(dst) + ".tmp")
        if tmp.exists():
            shutil.rmtree(tmp)
        shutil.copytree(src_dir, tmp)
        info = json.loads((tmp / "act_info.json").read_text())
        for ent in info["act_func_sets"]:
            if "exp" not in ent["act"]:
                continue
            prof = json.loads((tmp / ent["profile_json"]).read_text())
            starts = prof["func_to_bkt_start_idx"]
            s0 = starts["exp"]
            later = [v for v in starts.values() if v > s0]
            s1_ = min(later) if later else prof["bkt_entry_cnt"]
            binp = tmp / ent["bkt_bin"]
            tbl = np.fromfile(binp, dtype=np.float32).reshape(-1, 8)
            seg = tbl[s0:s1_]
            x = seg[:, 4].astype(np.float64)
            neg = (x < 0) & ~((seg[:, 0] == 0) & (seg[:, 1] == 0))
            h = np.exp(ALPHA * x[neg])
            seg[neg, 0] = h
            seg[neg, 1] = ALPHA * h
            seg[neg, 2] = (ALPHA**2 / 2.0) * h
            seg[neg, 3] = (ALPHA**3 / 6.0) * h
            tbl[s0:s1_] = seg
            tbl.tofile(binp)
        os.rename(tmp, dst)
    _CUSTOM["act_root"] = str(info_path)
    return str(info_path)


_CUSTOM = {}

N_AFF = 12  # tiles using the raw u8 mask via the 1x DVE affine op


def tile_split(n_aff=N_AFF):
    """Returns (tt_tiles, aff_tiles): aff tiles (u8 mask, 1x DVE op) are
    spread evenly; tt tiles (fp16 mask, 2x tensor_tensor) fill the rest."""
    aff = sorted({(i * NT) // n_aff for i in range(n_aff)}) if n_aff else []
    tt = [t for t in range(NT) if t not in set(aff)]
    return tt, aff


def _register_mask_affine():
    """One fused VectorE op (1x): w = m*imm2 + s2b + s1[i] from the raw u8
    mask -- the pre-exp affine for tiles whose mask stays u8 in DRAM."""
    if "aff" in _CUSTOM:
        return _CUSTOM["aff"]
    from concourse import dve_ops
    from concourse.dve_spec import C0, C1, C2, Spec, Src0, Src1, _has_src1, lower
    from concourse.dve_uop import DveOpSpec

    name = "MASK_AFFINE_ANT_X"

    def _ref(in0, in1, c0, c1, c2):
        import numpy as np_

        return (in0.astype(np_.float32) * c2 + in1 + c0).astype(np_.float32)

    spec = Spec(body=Src0 * C2 + Src1 + C0, reference=_ref)
    row = dve_ops._CUSTOM_DVE_ROW_BASE + len(dve_ops.OPS)
    uops = lower(spec, ver="v3")
    sha = DveOpSpec(
        name=name, opcode=row, uops=uops, rd1_en=_has_src1(spec)
    ).sha("v3")
    op = dve_ops.DveOp(name, spec, subdim=False, uops_sha={"v3": sha})
    dve_ops.OPS.append(op)
    dve_ops.CUSTOM_DVE_SPECS[name] = spec
    dve_ops._SUB_OPCODE_FOR_NAME[name] = row
    _CUSTOM["aff"] = op
    return op


def _register_mask_leaky():
    """One fused VectorE op: u = max(5*y, y), y = m*imm2 + s2b + s1[i].
    5*leakyrelu(y) with the mask fill folded in; exp applies scale=0.2.
    Reads the raw u8 mask directly (the op runs at 1x regardless of dtype)."""
    if "u" in _CUSTOM:
        return _CUSTOM["u"]
    from concourse import dve_ops
    from concourse.dve_spec import C0, C1, C2, Spec, Src0, Src1, _has_src1, lower, maxx
    from concourse.dve_uop import DveOpSpec

    name = "MASK_LEAKY_ANT_X"
    y = Src0 * C2 + Src1 + C0

    def _ref(in0, in1, c0, c1, c2):
        import numpy as np_

        yy = in0.astype(np_.float32) * c2 + in1 + c0
        return np_.maximum(yy * c1, yy).astype(np_.float32)

    spec = Spec(body=maxx(y * C1, y), reference=_ref)
    row = dve_ops._CUSTOM_DVE_ROW_BASE + len(dve_ops.OPS)
    uops = lower(spec, ver="v3")
    sha = DveOpSpec(
        name=name, opcode=row, uops=uops, rd1_en=_has_src1(spec)
    ).sha("v3")
    op = dve_ops.DveOp(name, spec, subdim=False, uops_sha={"v3": sha})
    dve_ops.OPS.append(op)
    dve_ops.CUSTOM_DVE_SPECS[name] = spec
    dve_ops._SUB_OPCODE_FOR_NAME[name] = row
    _CUSTOM["u"] = op
    return op


def build(n_aff=N_AFF, out_dt_name="float16"):
    import os
    from contextlib import ExitStack

    import concourse.mybir as mybir
    import concourse.tile as tile
    from concourse import bacc

    dt = mybir.dt
    Act = mybir.ActivationFunctionType
    cdt = dt.float16
    odt = getattr(dt, out_dt_name)

    os.environ["BASS_ACT_ROOT_JSON_PATH"] = _make_hijacked_act_root()
    mask_affine = _register_mask_affine()
    tt_tiles, aff_tiles = tile_split(n_aff)
    aff_set = set(aff_tiles)

    nc = bacc.Bacc("TRN2", target_bir_lowering=False, debug=False, num_devices=8)
    xt_ext = nc.dram_tensor("xt", [F, N], cdt, kind="ExternalInput").ap()
    m16_ext = nc.dram_tensor(
        "mask16", [max(len(tt_tiles), 1) * P, N], dt.float16, kind="ExternalInput"
    ).ap()
    m8_ext = nc.dram_tensor(
        "mask8", [max(len(aff_tiles), 1) * P, N], dt.uint8, kind="ExternalInput"
    ).ap()
    w_ext = nc.dram_tensor("w", [F, 2], cdt, kind="ExternalInput").ap()
    out_ext = nc.dram_tensor("out", [N, N], odt, kind="ExternalOutput").ap()
    m16_row = {t: i for i, t in enumerate(tt_tiles)}
    m8_row = {t: i for i, t in enumerate(aff_tiles)}

    with tile.TileContext(nc) as tc, ExitStack() as ctx:
        persist = ctx.enter_context(tc.tile_pool(name="persist", bufs=1))
        psum = ctx.enter_context(tc.tile_pool(name="psum", bufs=1, space="PSUM"))

        s1col = persist.tile([P, NT], dt.float32, tag="s1col")
        s1colh = persist.tile([P, NT], cdt, tag="s1colh")
        s2b = persist.tile([P, N], cdt, tag="s2b")
        xt_sb = persist.tile([P, 2, N], cdt, tag="xt")
        w_sb = persist.tile([P, 2, 2], cdt, tag="w")
        s12h = persist.tile([2, N], cdt, tag="s12h")

        CH = 512
        NJ = N // CH
        s12d = nc.dram_tensor("s12scratch", [2, N], cdt).ap()
        # xt in quarter chunks so the projection matmuls pipeline behind the DMA
        XQ = N // 4
        xt_dmas = []
        for q in range(4):
            for a in range(2):
                xd = nc.sync.dma_start(
                    xt_sb[:, a, q * XQ : (q + 1) * XQ],
                    xt_ext[a * P : (a + 1) * P, q * XQ : (q + 1) * XQ],
                )
                xt_dmas.append(xd.ins)
            if q == 0:
                for a in range(2):
                    nc.sync.dma_start(w_sb[:, a, :], w_ext[a * P : (a + 1) * P, :])

        # s12 = [s2; s1] rows via thin [2, CH] matmuls; psum->sbuf casts on
        # DVE (prologue-critical-path work while DVE has nothing else to do)
        for j in range(NJ):
            ps = psum.tile([2, CH], dt.float32, tag=f"ps{j % 4}", name=f"pss{j}")
            for a in range(2):
                nc.tensor.matmul(
                    ps[:],
                    w_sb[:, a, :],
                    xt_sb[:, a, j * CH : (j + 1) * CH],
                    start=(a == 0),
                    stop=(a == 1),
                )
            nc.vector.tensor_copy(s12h[:, j * CH : (j + 1) * CH], ps[:])
        def finish_prologue():
            # round-trip through DRAM: s1col via strided readback, s2b via
            # outer-stride-0 broadcast readback (replaces PE broadcast matmuls)
            nc.sync.dma_start(s12d[:], s12h[:])
            nc.sync.dma_start(s1colh[:], s12d[1, :].rearrange("(t p) -> p t", p=P))
            nc.vector.tensor_copy(s1col[:], s1colh[:])
            s2b_bc = s12d[0:1, :].partition_broadcast(P).squeeze(1)
            nc.sync.dma_start(s2b[:], s2b_bc)

        mp16 = ctx.enter_context(tc.tile_pool(name="mask16p", bufs=5))
        mp8 = ctx.enter_context(tc.tile_pool(name="mask8p", bufs=4))
        wp = ctx.enter_context(tc.tile_pool(name="work", bufs=3))
        pp = ctx.enter_context(tc.tile_pool(name="prob", bufs=4))
        op = ctx.enter_context(tc.tile_pool(name="outp", bufs=3))
        rp = ctx.enter_context(tc.tile_pool(name="redu", bufs=6))

        DLY = 3   # recip/normalize run this many tiles behind the exp pipeline
        LEAD = 6  # mask DMA issues run this many tiles ahead of compute
        p_tiles, r_tiles = {}, {}
        m_tiles = {}

        def mask_load(t):
            """Issue tile t's mask DMA on the Sync queue."""
            if t in aff_set:
                i8 = m8_row[t]
                m_sb = mp8.tile([P, N], dt.uint8, tag="m8")
                nc.sync.dma_start(m_sb[:], m8_ext[i8 * P : (i8 + 1) * P, :])
            else:
                i16 = m16_row[t]
                m_sb = mp16.tile([P, N], cdt, tag="m16")
                nc.sync.dma_start(m_sb[:], m16_ext[i16 * P : (i16 + 1) * P, :])
            m_tiles[t] = m_sb

        def front(t):
            p_t = pp.tile([P, N], cdt, tag="p")
            r_t = rp.tile([P, 1], dt.float32, tag="r")
            p_tiles[t], r_tiles[t] = p_t, r_t
            m_sb = m_tiles.pop(t)
            w_t = wp.tile([P, N], cdt, tag="wu", name="w_t")
            if t in aff_set:
                # u8 mask: one 1x DVE op w = -100*m + s2b + s1[i]
                nc.vector._custom_dve(
                    mask_affine,
                    out=w_t[:],
                    in0=m_sb[:],
                    in1=s2b[:],
                    s0=s1col[:, t : t + 1],
                    s1=1.0,
                    imm2=MASKC,
                )
                bias = 0.0
            else:
                # fp16 {-100,0} mask: 2x tensor_tensor; s1 rides Exp's bias
                nc.vector.tensor_add(w_t[:], m_sb[:], s2b[:])
                bias = s1col[:, t : t + 1]
            # hijacked Exp table computes exp(leakyrelu(.)) with fused rowsum
            nc.scalar.activation(
                p_t[:], w_t[:], Act.Exp, bias=bias, scale=1.0, accum_out=r_t[:]
            )

        def back(t):
            p_t, r_t = p_tiles.pop(t), r_tiles.pop(t)
            rec = rp.tile([P, 1], dt.float32, tag="rec")
            nc.vector.reciprocal(rec[:], r_t[:])
            o_t = op.tile([P, N], odt, tag="o")
            nc.vector.tensor_scalar_mul(o_t[:], p_t[:], rec[:, 0:1])
            nc.sync.dma_start(out_ext[t * P : (t + 1) * P, :], o_t[:])

        # mask DMAs for the first LEAD tiles go out on the Sync queue before
        # the prologue's s12d round-trip (whose issues wait on the matmuls);
        # the transfers overlap the projection pipeline.
        for t in range(LEAD):
            mask_load(t)
        finish_prologue()
        for t in range(NT):
            if t + LEAD < NT:
                mask_load(t + LEAD)
            front(t)
            if t >= DLY:
                back(t - DLY)
        for t in range(NT - DLY, NT):
            back(t)

    nc.compile()
    return nc


def make_in_maps(x, mask, w1, w2, n_aff=N_AFF):
    tt_tiles, aff_tiles = tile_split(n_aff)
    x = np.asarray(x, dtype=np.float32)
    mask = np.asarray(mask)
    mview = mask.reshape(B, NT, P, N)
    w = np.ascontiguousarray(
        np.stack([np.asarray(w2, np.float16), np.asarray(w1, np.float16)], axis=1)
    )
    in_maps = []
    for b in range(B):
        if tt_tiles:
            m16 = np.where(
                mview[b, tt_tiles], np.float16(MASKC), np.float16(0.0)
            ).reshape(len(tt_tiles) * P, N)
        else:
            m16 = np.zeros((P, N), np.float16)
        if aff_tiles:
            m8 = np.ascontiguousarray(
                mview[b, aff_tiles].reshape(len(aff_tiles) * P, N).astype(np.uint8)
            )
        else:
            m8 = np.zeros((P, N), np.uint8)
        in_maps.append(
            {
                "xt": np.ascontiguousarray(x[b].T.astype(np.float16)),
                "mask16": m16,
                "mask8": m8,
                "w": w,
            }
        )
    return in_maps


def kernel(x, mask, w1, w2, trace=False, nc=None, n_aff=N_AFF):
    from concourse.bass_utils import run_bass_kernel_spmd

    if trace:
        _install_ntff_hook()
    if nc is None:
        nc = build(n_aff)
    in_maps = make_in_maps(x, mask, w1, w2, n_aff)
    res = run_bass_kernel_spmd(nc, in_maps, core_ids=list(range(B)), trace=trace)
    out = np.stack(
        [np.asarray(res.results[b]["out"]).astype(np.float32) for b in range(B)]
    )
    kernel.last_result = res
    return out


def _install_ntff_hook():
    import sys
    import types

    if "antenv.axon_hooks" in sys.modules:
        return
    from trn_agent_boot.trn_boot import _ntff_profile_via_ctypes

    hook = _ntff_profile_via_ctypes("/opt/axon/libaxon_pjrt.so")
    mod = types.ModuleType("antenv.axon_hooks")
    mod.get_axon_ntff_profile_hook = lambda: hook
    mod.set_axon_ntff_profile_hook = lambda h: None
    sys.modules["antenv.axon_hooks"] = mod
    import antenv

    antenv.axon_hooks = mod


# revision 18
# speedup vs baseline: 1.2148x; 1.2148x over previous
"""Trainium2 Bass kernel for masked GAT-style attention softmax.

reference: softmax(where(mask, -1e9, leakyrelu(s1[:,None]+s2[None,:])), -1)
with s1 = x@w1, s2 = x@w2.  B=8 batches -> data-parallel over 8 NeuronCores.

Per-core layout [i_part, j_free], fp16 compute / f32 accums.  The ACT
engine's exp lookup table is rebuilt at compile time so that for x < 0 it
evaluates exp(alpha*x) instead of exp(x) -- i.e. Exp becomes
H(x) = exp(leakyrelu(x)) in a single table pass (the PWP bucket tables
store per-segment Taylor coefficients [d0..d3, x0]; only the negative
segments' coefficients change, positives and saturation stay stock).

Per tile [128, 4096]:
  DVE : w = m16 + s2b            (tensor_tensor, 2x mode, fp16)
  ACT : p = H(w + s1[i]), rowsum (hijacked Exp, per-partition bias, accum)
  DVE : r -> 1/r (reciprocal), out = p * (1/r) (tensor_scalar, 4x mode)
ACT is the bottleneck at ~137us/core; DVE ~128us.  GpSimd only issues
output DMAs (Pool tensor ops measured 10-50x below the cost model on HW
and they stall DVE via the shared SBUF ports -- keep Pool off compute).
Masks + prologue loads issue on Sync; mask issues run LEAD tiles ahead.
s2b comes from a stride-0 broadcast DMA readback of the s12 projection
row.  Host pre-bakes the fp16 {-100, 0} mask fill."""

import numpy as np

B, N, F = 8, 4096, 256
P = 128
NT = N // P  # 32 row tiles per core
MASKC = -100.0
ALPHA = 0.2


def _make_hijacked_act_root():
    """Build a patched copy of the neuronxcc PWP activation tables where
    exp's negative-x bucket entries hold Taylor coefficients of
    exp(ALPHA*x), so ActivationFunctionType.Exp computes exp(leakyrelu(x)).
    Returns the path to the patched act_info.json (cached per-process)."""
    import hashlib
    import json
    import os
    import shutil
    from pathlib import Path

    if _CUSTOM.get("act_root"):
        return _CUSTOM["act_root"]

    from neuronxcc.driver.Job import Job

    pkg = Path(Job.getPackageDir())
    src_dir = None
    for cand in ("pwp_bin_trainium",):
        if (pkg / "pwp" / cand / "act_info.json").exists():
            src_dir = pkg / "pwp" / cand
    if src_dir is None:
        from neuronxcc.driver.jobs.support.FindActInfo import findActInfoFile

        src_dir = Path(findActInfoFile(str(pkg), "gen3")).parent

    tag = hashlib.md5(
        f"lrelu-exp-{ALPHA}-{src_dir}".encode()
    ).hexdigest()[:10]
    dst = Path(os.environ.get("TMPDIR", "/tmp")) / f"bass_act_lrelu_{tag}"
    info_path = dst / "act_info.json"
    if not info_path.exists():
        tmp = Path(str(dst) + ".tmp")
        if tmp.exists():
            shutil.rmtree(tmp)
        shutil.copytree(src_dir, tmp)
        info = json.loads((tmp / "act_info.json").read_text())
        for ent in info["act_func_sets"]:
            if "exp" not in ent["act"]:
                continue
            prof = json.loads((tmp / ent["profile_json"]).read_text())
            starts = prof["func_to_bkt_start_idx"]
            s0 = starts["exp"]
            later = [v for v in starts.values() if v > s0]
            s1_ = min(later) if later else prof["bkt_entry_cnt"]
            binp = tmp / ent["bkt_bin"]
            tbl = np.fromfile(binp, dtype=np.float32).reshape(-1, 8)
            seg = tbl[s0:s1_]
            x = seg[:, 4].astype(np.float64)
            neg = (x < 0) & ~((seg[:, 0] == 0) & (seg[:, 1] == 0))
            h = np.exp(ALPHA * x[neg])
            seg[neg, 0] = h
            seg[neg, 1] = ALPHA * h
            seg[neg, 2] = (ALPHA**2 / 2.0) * h
            seg[neg, 3] = (ALPHA**3 / 6.0) * h
            tbl[s0:s1_] = seg
            tbl.tofile(binp)
        os.rename(tmp, dst)
    _CUSTOM["act_root"] = str(info_path)
    return str(info_path)


_CUSTOM = {}

N_AFF = 12  # tiles using the raw u8 mask via the 1x DVE affine op


def tile_split(n_aff=N_AFF):
    """Returns (tt_tiles, aff_tiles): aff tiles (u8 mask, 1x DVE op) are
    spread evenly; tt tiles (fp16 mask, 2x tensor_tensor) fill the rest."""
    aff = sorted({(i * NT) // n_aff for i in range(n_aff)}) if n_aff else []
    tt = [t for t in range(NT) if t not in set(aff)]
    return tt, aff


def _register_mask_affine():
    """One fused VectorE op (1x): w = m*imm2 + s2b + s1[i] from the raw u8
    mask -- the pre-exp affine for tiles whose mask stays u8 in DRAM."""
    if "aff" in _CUSTOM:
        return _CUSTOM["aff"]
    from concourse import dve_ops
    from concourse.dve_spec import C0, C1, C2, Spec, Src0, Src1, _has_src1, lower
    from concourse.dve_uop import DveOpSpec

    name = "MASK_AFFINE_ANT_X"

    def _ref(in0, in1, c0, c1, c2):
        import numpy as np_

        return (in0.astype(np_.float32) * c2 + in1 + c0).astype(np_.float32)

    spec = Spec(body=Src0 * C2 + Src1 + C0, reference=_ref)
    row = dve_ops._CUSTOM_DVE_ROW_BASE + len(dve_ops.OPS)
    uops = lower(spec, ver="v3")
    sha = DveOpSpec(
        name=name, opcode=row, uops=uops, rd1_en=_has_src1(spec)
    ).sha("v3")
    op = dve_ops.DveOp(name, spec, subdim=False, uops_sha={"v3": sha})
    dve_ops.OPS.append(op)
    dve_ops.CUSTOM_DVE_SPECS[name] = spec
    dve_ops._SUB_OPCODE_FOR_NAME[name] = row
    _CUSTOM["aff"] = op
    return op


def _register_mask_leaky():
    """One fused VectorE op: u = max(5*y, y), y = m*imm2 + s2b + s1[i].
    5*leakyrelu(y) with the mask fill folded in; exp applies scale=0.2.
    Reads the raw u8 mask directly (the op runs at 1x regardless of dtype)."""
    if "u" in _CUSTOM:
        return _CUSTOM["u"]
    from concourse import dve_ops
    from concourse.dve_spec import C0, C1, C2, Spec, Src0, Src1, _has_src1, lower, maxx
    from concourse.dve_uop import DveOpSpec

    name = "MASK_LEAKY_ANT_X"
    y = Src0 * C2 + Src1 + C0

    def _ref(in0, in1, c0, c1, c2):
        import numpy as np_

        yy = in0.astype(np_.float32) * c2 + in1 + c0
        return np_.maximum(yy * c1, yy).astype(np_.float32)

    spec = Spec(body=maxx(y * C1, y), reference=_ref)
    row = dve_ops._CUSTOM_DVE_ROW_BASE + len(dve_ops.OPS)
    uops = lower(spec, ver="v3")
    sha = DveOpSpec(
        name=name, opcode=row, uops=uops, rd1_en=_has_src1(spec)
    ).sha("v3")
    op = dve_ops.DveOp(name, spec, subdim=False, uops_sha={"v3": sha})
    dve_ops.OPS.append(op)
    dve_ops.CUSTOM_DVE_SPECS[name] = spec
    dve_ops._SUB_OPCODE_FOR_NAME[name] = row
    _CUSTOM["u"] = op
    return op


def build(n_aff=N_AFF, out_dt_name="float16"):
    import os
    from contextlib import ExitStack

    import concourse.mybir as mybir
    import concourse.tile as tile
    from concourse import bacc

    dt = mybir.dt
    Act = mybir.ActivationFunctionType
    cdt = dt.float16
    odt = getattr(dt, out_dt_name)

    os.environ["BASS_ACT_ROOT_JSON_PATH"] = _make_hijacked_act_root()
    mask_affine = _register_mask_affine()
    tt_tiles, aff_tiles = tile_split(n_aff)
    aff_set = set(aff_tiles)

    nc = bacc.Bacc("TRN2", target_bir_lowering=False, debug=False, num_devices=8)
    xt_ext = nc.dram_tensor("xt", [F, N], cdt, kind="ExternalInput").ap()
    m16_ext = nc.dram_tensor(
        "mask16", [max(len(tt_tiles), 1) * P, N], dt.float16, kind="ExternalInput"
    ).ap()
    m8_ext = nc.dram_tensor(
        "mask8", [max(len(aff_tiles), 1) * P, N], dt.uint8, kind="ExternalInput"
    ).ap()
    w_ext = nc.dram_tensor("w", [F, 2], cdt, kind="ExternalInput").ap()
    out_ext = nc.dram_tensor("out", [N, N], odt, kind="ExternalOutput").ap()
    m16_row = {t: i for i, t in enumerate(tt_tiles)}
    m8_row = {t: i for i, t in enumerate(aff_tiles)}

    with tile.TileContext(nc) as tc, ExitStack() as ctx:
        persist = ctx.enter_context(tc.tile_pool(name="persist", bufs=1))
        psum = ctx.enter_context(tc.tile_pool(name="psum", bufs=1, space="PSUM"))

        s1col = persist.tile([P, NT], dt.float32, tag="s1col")
        s1colh = persist.tile([P, NT], cdt, tag="s1colh")
        s2b = persist.tile([P, N], cdt, tag="s2b")
        xt_sb = persist.tile([P, 2, N], cdt, tag="xt")
        w_sb = persist.tile([P, 2, 2], cdt, tag="w")
        s12h = persist.tile([2, N], cdt, tag="s12h")

        ones128 = persist.tile([1, P], cdt, tag="ones")
        CH = 512
        NJ = N // CH
        nc.vector.memset(ones128[:], 1.0)
        s1d = nc.dram_tensor("s1scratch", [1, N], cdt).ap()
        # xt in quarter chunks so the projection matmuls pipeline behind the DMA
        XQ = N // 4
        xt_dmas = []
        for q in range(4):
            for a in range(2):
                xd = nc.sync.dma_start(
                    xt_sb[:, a, q * XQ : (q + 1) * XQ],
                    xt_ext[a * P : (a + 1) * P, q * XQ : (q + 1) * XQ],
                )
                xt_dmas.append(xd.ins)
            if q == 0:
                for a in range(2):
                    nc.sync.dma_start(w_sb[:, a, :], w_ext[a * P : (a + 1) * P, :])

        # s12 = [s2; s1] rows via thin [2, CH] matmuls; psum->sbuf casts on
        # DVE (prologue-critical-path work while DVE has nothing else to do)
        for j in range(NJ):
            ps = psum.tile([2, CH], dt.float32, tag=f"ps{j % 4}", name=f"pss{j}")
            for a in range(2):
                nc.tensor.matmul(
                    ps[:],
                    w_sb[:, a, :],
                    xt_sb[:, a, j * CH : (j + 1) * CH],
                    start=(a == 0),
                    stop=(a == 1),
                )
            nc.vector.tensor_copy(s12h[:, j * CH : (j + 1) * CH], ps[:])
        def finish_prologue():
            # s1col via DRAM strided readback; s2b via rank-1 PE broadcast
            # matmuls (chunked through PSUM, casts on DVE -- all prologue
            # dead time, overlapped with the xt/mask DMAs)
            nc.sync.dma_start(s1d[:], s12h[1:2, :])
            nc.sync.dma_start(s1colh[:], s1d[0, :].rearrange("(t p) -> p t", p=P))
            nc.vector.tensor_copy(s1col[:], s1colh[:])
            for j in range(NJ):
                psb = psum.tile([P, CH], dt.float32, tag=f"psb{j % 2}", name=f"psb{j}")
                nc.tensor.matmul(
                    psb[:], ones128[:], s12h[0:1, j * CH : (j + 1) * CH],
                    start=True, stop=True,
                )
                nc.vector.tensor_copy(s2b[:, j * CH : (j + 1) * CH], psb[:])

        mp16 = ctx.enter_context(tc.tile_pool(name="mask16p", bufs=5))
        mp8 = ctx.enter_context(tc.tile_pool(name="mask8p", bufs=4))
        wp = ctx.enter_context(tc.tile_pool(name="work", bufs=3))
        pp = ctx.enter_context(tc.tile_pool(name="prob", bufs=4))
        op = ctx.enter_context(tc.tile_pool(name="outp", bufs=3))
        rp = ctx.enter_context(tc.tile_pool(name="redu", bufs=6))

        DLY = 3   # recip/normalize run this many tiles behind the exp pipeline
        LEAD = 6  # mask DMA issues run this many tiles ahead of compute
        p_tiles, r_tiles = {}, {}
        m_tiles = {}

        def mask_load(t):
            """fp16 masks ride the Sync HWDGE queue; u8 masks ride the GpSimd
            SWDGE queue (also keeps the Pool sequencer warm for out DMAs)."""
            if t in aff_set:
                i8 = m8_row[t]
                m_sb = mp8.tile([P, N], dt.uint8, tag="m8")
                nc.gpsimd.dma_start(m_sb[:], m8_ext[i8 * P : (i8 + 1) * P, :])
            else:
                i16 = m16_row[t]
                m_sb = mp16.tile([P, N], cdt, tag="m16")
                nc.sync.dma_start(m_sb[:], m16_ext[i16 * P : (i16 + 1) * P, :])
            m_tiles[t] = m_sb

        def front(t):
            p_t = pp.tile([P, N], cdt, tag="p")
            r_t = rp.tile([P, 1], dt.float32, tag="r")
            p_tiles[t], r_tiles[t] = p_t, r_t
            m_sb = m_tiles.pop(t)
            w_t = wp.tile([P, N], cdt, tag="wu", name="w_t")
            if t in aff_set:
                # u8 mask: one 1x DVE op w = -100*m + s2b + s1[i]
                nc.vector._custom_dve(
                    mask_affine,
                    out=w_t[:],
                    in0=m_sb[:],
                    in1=s2b[:],
                    s0=s1col[:, t : t + 1],
                    s1=1.0,
                    imm2=MASKC,
                )
                bias = 0.0
            else:
                # fp16 {-100,0} mask: 2x tensor_tensor; s1 rides Exp's bias
                nc.vector.tensor_add(w_t[:], m_sb[:], s2b[:])
                bias = s1col[:, t : t + 1]
            # hijacked Exp table computes exp(leakyrelu(.)) with fused rowsum
            nc.scalar.activation(
                p_t[:], w_t[:], Act.Exp, bias=bias, scale=1.0, accum_out=r_t[:]
            )

        def back(t):
            p_t, r_t = p_tiles.pop(t), r_tiles.pop(t)
            rec = rp.tile([P, 1], dt.float32, tag="rec")
            nc.vector.reciprocal(rec[:], r_t[:])
            o_t = op.tile([P, N], odt, tag="o")
            nc.vector.tensor_scalar_mul(o_t[:], p_t[:], rec[:, 0:1])
            # ~1/4 of outputs on the Sync queue evens the per-queue bytes
            eng = nc.sync if t % 4 == 1 else nc.gpsimd
            eng.dma_start(out_ext[t * P : (t + 1) * P, :], o_t[:])

        # mask DMAs for the first LEAD tiles go out on the Sync queue before
        # the prologue's s12d round-trip (whose issues wait on the matmuls);
        # the transfers overlap the projection pipeline.
        for t in range(LEAD):
            mask_load(t)
        finish_prologue()
        for t in range(NT):
            if t + LEAD < NT:
                mask_load(t + LEAD)
            front(t)
            if t >= DLY:
                back(t - DLY)
        for t in range(NT - DLY, NT):
            back(t)

    nc.compile()
    return nc


def make_in_maps(x, mask, w1, w2, n_aff=N_AFF):
    tt_tiles, aff_tiles = tile_split(n_aff)
    x = np.asarray(x, dtype=np.float32)
    mask = np.asarray(mask)
    mview = mask.reshape(B, NT, P, N)
    w = np.ascontiguousarray(
        np.stack([np.asarray(w2, np.float16), np.asarray(w1, np.float16)], axis=1)
    )
    in_maps = []
    for b in range(B):
        if tt_tiles:
            m16 = np.where(
                mview[b, tt_tiles], np.float16(MASKC), np.float16(0.0)
            ).reshape(len(tt_tiles) * P, N)
        else:
            m16 = np.zeros((P, N), np.float16)
        if aff_tiles:
            m8 = np.ascontiguousarray(
                mview[b, aff_tiles].reshape(len(aff_tiles) * P, N).astype(np.uint8)
            )
        else:
            m8 = np.zeros((P, N), np.uint8)
        in_maps.append(
            {
                "xt": np.ascontiguousarray(x[b].T.astype(np.float16)),
                "mask16": m16,
                "mask8": m8,
                "w": w,
            }
        )
    return in_maps


def kernel(x, mask, w1, w2, trace=False, nc=None, n_aff=N_AFF):
    from concourse.bass_utils import run_bass_kernel_spmd

    if trace:
        _install_ntff_hook()
    if nc is None:
        nc = build(n_aff)
    in_maps = make_in_maps(x, mask, w1, w2, n_aff)
    res = run_bass_kernel_spmd(nc, in_maps, core_ids=list(range(B)), trace=trace)
    out = np.stack(
        [np.asarray(res.results[b]["out"]).astype(np.float32) for b in range(B)]
    )
    kernel.last_result = res
    return out


def _install_ntff_hook():
    import sys
    import types

    if "antenv.axon_hooks" in sys.modules:
        return
    from trn_agent_boot.trn_boot import _ntff_profile_via_ctypes

    hook = _ntff_profile_via_ctypes("/opt/axon/libaxon_pjrt.so")
    mod = types.ModuleType("antenv.axon_hooks")
    mod.get_axon_ntff_profile_hook = lambda: hook
    mod.set_axon_ntff_profile_hook = lambda h: None
    sys.modules["antenv.axon_hooks"] = mod
    import antenv

    antenv.axon_hooks = mod
